# revision 2
# baseline (speedup 1.0000x reference)
"""Trainium2 Bass kernel for the nn_Decoder LSTM problem.

Teacher-forced LSTM decoder, T=8192 steps, D=1024, with the quirk that the
candidate-cell gate reads [h, c] instead of [h, x].

Strategy
--------
The sequential recurrence is solved by a scan-accelerated fixed-point
(Picard/Gauss-Seidel) iteration instead of stepping 8192 times:

  - Shard time across 8 cores: core k owns output rows [1024k, 1024k+1024).
    Each core processes a chunk of L=1152 steps (128 warm-up steps with a
    zero boundary state; the LSTM's fading memory makes the boundary error
    decay below 1e-6 within ~128 steps).  Zero cross-core communication.
  - Per sweep, gate pre-activations for all timesteps in the chunk are big
    dense matmuls against the previous iterate of (h, c) in [d, t] layout.
  - Given the gates, the c-recurrence c_t = f_t*c_{t-1} + i_t*ctilde_t is
    linear-diagonal and solved EXACTLY with the DVE tensor_tensor_scan op
    (fp32 internal state).  One inner refinement re-solves c with the
    updated ctilde.  This converges at ~0.63x error per sweep; K sweeps
    reach the fp16 fixed-point plateau (~6e-4 rel L2 vs the fp32 scan).
  - x-contributions of gates f,i,o are precomputed once (phase 1) and
    streamed from a DRAM scratch per sweep.

Everything on-chip is [d (partition), t (free)] so matmul outputs, the
elementwise chain, the scan, and the next sweep's matmul inputs all share
one layout; no transposes anywhere on device.
"""

import sys
import numpy as np

for _p in ("/opt/trn_rl_repo", "/root/.axon_site/_ro/trn_rl_repo"):
    if _p not in sys.path:
        sys.path.insert(0, _p)

import concourse.bass as bass
import concourse.bacc as bacc
import concourse.mybir as mybir
import concourse.tile as tile
from concourse.bass_utils import run_bass_kernel_spmd

D = 1024
T = 8192
KC = 8            # contraction chunks (1024/128)
DELTA = 128       # warm-up overlap steps
L = 1024 + DELTA  # chunk length per core
N_CORES = 8
K_SWEEPS = 18     # fixed-point sweeps (fp16 plateau is reached at ~18)

F16 = mybir.dt.float16
F32 = mybir.dt.float32
AF = mybir.ActivationFunctionType
ALU = mybir.AluOpType

# time-tiles per sweep: (t0, N)
M_TILES = [(0, 512), (512, 512), (1024, 128)]


def build_nc(k_sweeps: int = K_SWEEPS, trace_scopes: bool = False):
    nc = bacc.Bacc(None, target_bir_lowering=False, debug=False)

    # ---- I/O ----
    wh_t = nc.declare_dram_parameter("wh_t", [D, 4 * D], F16, isOutput=False)
    wc_t = nc.declare_dram_parameter("wc_t", [D, D], F16, isOutput=False)
    wx_t = nc.declare_dram_parameter("wx_t", [D, 3 * D], F16, isOutput=False)
    x_t = nc.declare_dram_parameter("x_t", [D, L], F16, isOutput=False)
    bias_f = nc.declare_dram_parameter("bias_f", [128, KC], F32, isOutput=False)
    bias_i = nc.declare_dram_parameter("bias_i", [128, KC], F32, isOutput=False)
    bias_o = nc.declare_dram_parameter("bias_o", [128, KC], F32, isOutput=False)
    bias_c = nc.declare_dram_parameter("bias_c", [128, KC], F32, isOutput=False)
    hb = nc.declare_dram_parameter("hb", [128, KC, 1], F16, isOutput=False)
    cb = nc.declare_dram_parameter("cb", [128, KC, 1], F16, isOutput=False)
    ident = nc.declare_dram_parameter("ident", [128, 128], F16, isOutput=False)
    h_out = nc.declare_dram_parameter("h_out", [128, KC, L + 1], F16, isOutput=True)

    # x-contribution of f,i,o gates, [gate*8+gd, 128, L], computed in phase 1
    pre_dram = nc.dram_tensor("pre_scratch", [24, 128, L], F16)

    with tile.TileContext(nc) as tc:
        with (
            tc.tile_pool(name="const", bufs=1) as constp,
            tc.tile_pool(name="psum", bufs=8, space="PSUM") as psum,
        ):
            # constants
            bf_sb = constp.tile([128, KC], F32, tag="bf")
            bi_sb = constp.tile([128, KC], F32, tag="bi")
            bo_sb = constp.tile([128, KC], F32, tag="bo")
            bc_sb = constp.tile([128, KC], F32, tag="bc")
            id_sb = constp.tile([128, 128], F16, tag="ident")
            nc.sync.dma_start(bf_sb[:, :], bias_f[:, :])
            nc.sync.dma_start(bi_sb[:, :], bias_i[:, :])
            nc.sync.dma_start(bo_sb[:, :], bias_o[:, :])
            nc.sync.dma_start(bc_sb[:, :], bias_c[:, :])
            nc.sync.dma_start(id_sb[:, :], ident[:, :])

            # ---------------- phase 1: pre = Wx @ x  ----------------
            with tc.tile_pool(name="ph1", bufs=1) as ph1:
                xT_sb = ph1.tile([128, KC, L], F16, tag="xT")
                nc.sync.dma_start(
                    xT_sb[:, :, :], x_t[:, :].rearrange("(c p) t -> p c t", p=128)
                )
                for g in range(3):  # f, i, o
                    wx_sb = ph1.tile([128, KC, D], F16, tag="wxslab")
                    nc.sync.dma_start(
                        wx_sb[:, :, :],
                        wx_t[:, g * D:(g + 1) * D].rearrange(
                            "(c p) m -> p c m", p=128
                        ),
                    )
                    for (t0, N) in M_TILES:
                        for gd in range(KC):
                            ps = psum.tile([128, N], F32, tag="ps")
                            for kc in range(KC):
                                nc.tensor.matmul(
                                    ps[:, :],
                                    wx_sb[:, kc, gd * 128:(gd + 1) * 128],
                                    xT_sb[:, kc, t0:t0 + N],
                                    start=(kc == 0),
                                    stop=(kc == KC - 1),
                                )
                            pre16 = ph1.tile([128, N], F16, tag="pre16")
                            nc.vector.tensor_copy(pre16[:, :], ps[:, :])
                            nc.sync.dma_start(
                                pre_dram[g * KC + gd, :, t0:t0 + N], pre16[:, :]
                            )

            # ---------------- persistent state ----------------
            with (
                tc.tile_pool(name="weights", bufs=1) as wpool,
                tc.tile_pool(name="state", bufs=1) as spool,
                tc.tile_pool(name="gates", bufs=1) as gpool,
                tc.tile_pool(name="work", bufs=3) as wk,
                tc.tile_pool(name="prestream", bufs=6) as prepool,
            ):
                wh_sb = wpool.tile([128, KC, 4 * D], F16, tag="wh")
                wc_sb = wpool.tile([128, KC, D], F16, tag="wc")
                nc.sync.dma_start(
                    wh_sb[:, :, :], wh_t[:, :].rearrange("(c p) m -> p c m", p=128)
                )
                nc.sync.dma_start(
                    wc_sb[:, :, :], wc_t[:, :].rearrange("(c p) m -> p c m", p=128)
                )

                # h/c history, col j = value at local time j-1 (col 0 = boundary)
                Hf = spool.tile([128, KC, L + 1], F16, tag="H")
                Cf = spool.tile([128, KC, L + 1], F16, tag="C")
                nc.vector.memset(Hf[:, :, :], 0.0)
                nc.vector.memset(Cf[:, :, :], 0.0)
                nc.sync.dma_start(Hf[:, :, 0:1], hb[:, :, :])
                nc.sync.dma_start(Cf[:, :, 0:1], cb[:, :, :])

                def sweep_body():
                    for (t0, N) in M_TILES:
                        # persistent-for-this-m-tile gate tiles
                        f_all = gpool.tile([128, KC, 512], F16, tag="f")
                        i_all = gpool.tile([128, KC, 512], F16, tag="i")
                        o_all = gpool.tile([128, KC, 512], F16, tag="o")
                        zA = gpool.tile([128, KC, 512], F32, tag="zA")

                        # prefetch pre tiles for this m-tile
                        pf_t, pi_t, po_t = [], [], []
                        for gd in range(KC):
                            pf = prepool.tile([128, N], F16, tag="pf")
                            nc.sync.dma_start(pf[:, :], pre_dram[gd, :, t0:t0 + N])
                            pf_t.append(pf)
                        for gd in range(KC):
                            pi = prepool.tile([128, N], F16, tag="pi")
                            nc.sync.dma_start(pi[:, :], pre_dram[KC + gd, :, t0:t0 + N])
                            pi_t.append(pi)
                        for gd in range(KC):
                            po = prepool.tile([128, N], F16, tag="po")
                            nc.sync.dma_start(po[:, :], pre_dram[2 * KC + gd, :, t0:t0 + N])
                            po_t.append(po)

                        # ---- P1a: ctilde pre-activation parts (h-part kept in zA) ----
                        ct_tiles = []
                        for ch in range(KC):
                            psA = psum.tile([128, N], F32, tag="ps")
                            for kc in range(KC):
                                nc.tensor.matmul(
                                    psA[:, :],
                                    wh_sb[:, kc, 3 * D + ch * 128: 3 * D + (ch + 1) * 128],
                                    Hf[:, kc, t0:t0 + N],
                                    start=(kc == 0),
                                    stop=(kc == KC - 1),
                                )
                            nc.vector.tensor_copy(zA[:, ch, :N], psA[:, :])
                            psB = psum.tile([128, N], F32, tag="ps")
                            for kc in range(KC):
                                nc.tensor.matmul(
                                    psB[:, :],
                                    wc_sb[:, kc, ch * 128:(ch + 1) * 128],
                                    Cf[:, kc, t0:t0 + N],
                                    start=(kc == 0),
                                    stop=(kc == KC - 1),
                                )
                            tmp = wk.tile([128, N], F32, tag="tmp")
                            nc.vector.tensor_add(tmp[:, :], psB[:, :], zA[:, ch, :N])
                            ct = wk.tile([128, N], F16, tag="ct")
                            nc.scalar.activation(
                                ct[:, :], tmp[:, :], AF.Tanh, bias=bc_sb[:, ch:ch + 1]
                            )
                            ct_tiles.append(ct)

                        # ---- P1b: f and i gates ----
                        for gate_idx, (garr, pre_tiles, bias_sb) in enumerate(
                            [(f_all, pf_t, bf_sb), (i_all, pi_t, bi_sb)]
                        ):
                            for ch in range(KC):
                                ps = psum.tile([128, N], F32, tag="ps")
                                for kc in range(KC):
                                    nc.tensor.matmul(
                                        ps[:, :],
                                        wh_sb[:, kc, gate_idx * D + ch * 128: gate_idx * D + (ch + 1) * 128],
                                        Hf[:, kc, t0:t0 + N],
                                        start=(kc == 0),
                                        stop=False,
                                    )
                                nc.tensor.matmul(
                                    ps[:, :],
                                    id_sb[:, :],
                                    pre_tiles[ch][:, :],
                                    start=False,
                                    stop=True,
                                )
                                nc.scalar.activation(
                                    garr[:, ch, :N], ps[:, :], AF.Sigmoid,
                                    bias=bias_sb[:, ch:ch + 1],
                                )

                        # ---- scan 1: exact c given gates (writes Cf cols t0+1..) ----
                        for ch in range(KC):
                            b1 = wk.tile([128, N], F16, tag="b1")
                            nc.vector.tensor_mul(b1[:, :], i_all[:, ch, :N], ct_tiles[ch][:, :])
                            nc.vector.tensor_tensor_scan(
                                Cf[:, ch, t0 + 1:t0 + N + 1],
                                f_all[:, ch, :N],
                                b1[:, :],
                                Cf[:, ch, t0:t0 + 1],
                                ALU.mult,
                                ALU.add,
                            )

                        # ---- o gate (PE keeps busy while scans run) ----
                        for ch in range(KC):
                            ps = psum.tile([128, N], F32, tag="ps")
                            for kc in range(KC):
                                nc.tensor.matmul(
                                    ps[:, :],
                                    wh_sb[:, kc, 2 * D + ch * 128: 2 * D + (ch + 1) * 128],
                                    Hf[:, kc, t0:t0 + N],
                                    start=(kc == 0),
                                    stop=False,
                                )
                            nc.tensor.matmul(
                                ps[:, :], id_sb[:, :], po_t[ch][:, :],
                                start=False, stop=True,
                            )
                            nc.scalar.activation(
                                o_all[:, ch, :N], ps[:, :], AF.Sigmoid,
                                bias=bo_sb[:, ch:ch + 1],
                            )

                        # ---- P2: inner refinement of ctilde/c with updated c ----
                        ps2_tiles = []
                        for ch in range(KC):
                            ps2 = psum.tile([128, N], F32, tag="ps")
                            for kc in range(KC):
                                nc.tensor.matmul(
                                    ps2[:, :],
                                    wc_sb[:, kc, ch * 128:(ch + 1) * 128],
                                    Cf[:, kc, t0:t0 + N],
                                    start=(kc == 0),
                                    stop=(kc == KC - 1),
                                )
                            ps2_tiles.append(ps2)
                        for ch in range(KC):
                            tmp2 = wk.tile([128, N], F32, tag="tmp")
                            nc.vector.tensor_add(tmp2[:, :], ps2_tiles[ch][:, :], zA[:, ch, :N])
                            ct2 = wk.tile([128, N], F16, tag="ct")
                            nc.scalar.activation(
                                ct2[:, :], tmp2[:, :], AF.Tanh, bias=bc_sb[:, ch:ch + 1]
                            )
                            b2 = wk.tile([128, N], F16, tag="b1")
                            nc.vector.tensor_mul(b2[:, :], i_all[:, ch, :N], ct2[:, :])
                            nc.vector.tensor_tensor_scan(
                                Cf[:, ch, t0 + 1:t0 + N + 1],
                                f_all[:, ch, :N],
                                b2[:, :],
                                Cf[:, ch, t0:t0 + 1],
                                ALU.mult,
                                ALU.add,
                            )

                        # ---- P3: h = o * tanh(c) ----
                        for ch in range(KC):
                            tch = wk.tile([128, N], F16, tag="tch")
                            nc.scalar.activation(
                                tch[:, :], Cf[:, ch, t0 + 1:t0 + N + 1], AF.Tanh
                            )
                            nc.vector.tensor_mul(
                                Hf[:, ch, t0 + 1:t0 + N + 1], o_all[:, ch, :N], tch[:, :]
                            )

                with tc.For_i(0, k_sweeps, 1):
                    sweep_body()

                # ---------------- output ----------------
                nc.sync.dma_start(h_out[:, :, :], Hf[:, :, :])

    nc.compile()
    return nc


# ------------------------- host side -------------------------

def _prep_core_inputs(inputs):
    """Build the 8 per-core input maps from the full problem inputs."""
    x = np.asarray(inputs["target_seq"], np.float32)
    W_f = np.asarray(inputs["W_f"], np.float32)
    W_i = np.asarray(inputs["W_i"], np.float32)
    W_C = np.asarray(inputs["W_C"], np.float32)
    W_o = np.asarray(inputs["W_o"], np.float32)

    wh_t = np.concatenate(
        [W_f[:, :D].T, W_i[:, :D].T, W_o[:, :D].T, W_C[:, :D].T], axis=1
    ).astype(np.float16)                      # [D, 4D], cols = [f|i|o|C]
    wc_t = np.ascontiguousarray(W_C[:, D:].T).astype(np.float16)   # [D, D]
    wx_t = np.concatenate(
        [W_f[:, D:].T, W_i[:, D:].T, W_o[:, D:].T], axis=1
    ).astype(np.float16)                      # [D, 3D]

    def vec_pc(v):  # [D] -> [128, 8] with d = ch*128 + p
        return np.ascontiguousarray(np.asarray(v, np.float32).reshape(KC, 128).T)

    bias_f = vec_pc(inputs["b_f"])
    bias_i = vec_pc(inputs["b_i"])
    bias_o = vec_pc(inputs["b_o"])
    bias_c = vec_pc(inputs["b_C"])
    ident = np.eye(128, dtype=np.float16)

    h0 = np.asarray(inputs["encoder_h"], np.float32)
    c0 = np.asarray(inputs["encoder_c"], np.float32)

    in_maps = []
    for core in range(N_CORES):
        if core == 0:
            rows = slice(0, L)
            hb = vec_pc(h0).astype(np.float16)[:, :, None]
            cb = vec_pc(c0).astype(np.float16)[:, :, None]
        else:
            rows = slice(1024 * core - DELTA, 1024 * core + 1024)
            hb = np.zeros((128, KC, 1), np.float16)
            cb = np.zeros((128, KC, 1), np.float16)
        x_chunk_t = np.ascontiguousarray(x[rows].T).astype(np.float16)  # [D, L]
        in_maps.append({
            "wh_t": wh_t, "wc_t": wc_t, "wx_t": wx_t,
            "x_t": x_chunk_t,
            "bias_f": bias_f, "bias_i": bias_i, "bias_o": bias_o, "bias_c": bias_c,
            "hb": hb, "cb": cb, "ident": ident,
        })
    return in_maps


def _gather_output(results):
    """Assemble [T, D] fp32 from per-core h_out [128, 8, L+1] fp16."""
    out = np.empty((T, D), np.float32)
    for core in range(N_CORES):
        h = np.asarray(results[core]["h_out"])          # [128, 8, L+1] fp16
        # col j = h at local time j-1 ; d = ch*128 + p
        chunk = np.transpose(h, (2, 1, 0)).reshape(L + 1, D).astype(np.float32)
        if core == 0:
            out[0:1024] = chunk[1:1025]
        else:
            out[1024 * core:1024 * (core + 1)] = chunk[DELTA + 1:L + 1]
    return out


_NC_CACHE = {}


def _get_nc(k_sweeps=K_SWEEPS):
    if k_sweeps not in _NC_CACHE:
        _NC_CACHE[k_sweeps] = build_nc(k_sweeps)
    return _NC_CACHE[k_sweeps]


def kernel(**inputs) -> np.ndarray:
    nc = _get_nc()
    in_maps = _prep_core_inputs(inputs)
    res = run_bass_kernel_spmd(nc, in_maps, list(range(N_CORES)))
    return _gather_output(res.results)


if __name__ == "__main__":
    # smoke test with random data through the CoreSim interpreter is too slow;
    # use test.py against the reference instead.
    nc = build_nc()
    print("built ok")


# revision 8
# speedup vs baseline: 1.0557x; 1.0557x over previous
"""Trainium2 Bass kernel for the nn_Decoder LSTM problem.

Teacher-forced LSTM decoder, T=8192 steps, D=1024, with the quirk that the
candidate-cell gate reads [h, c] instead of [h, x].

Strategy
--------
The sequential recurrence is solved by a scan-accelerated fixed-point
(Picard/Gauss-Seidel) iteration instead of stepping 8192 times:

  - Shard time across 8 cores: core k owns output rows [1024k, 1024k+1024).
    Each core processes a chunk of L=1152 steps (128 warm-up steps with a
    zero boundary state; the LSTM's fading memory makes the boundary error
    decay below 1e-6 within ~128 steps).  Zero cross-core communication.
  - Per sweep, gate pre-activations for all timesteps in the chunk are big
    dense matmuls against the previous iterate of (h, c) in [d, t] layout.
  - Given the gates, the c-recurrence c_t = f_t*c_{t-1} + i_t*ctilde_t is
    linear-diagonal and solved EXACTLY with the DVE tensor_tensor_scan op
    (fp32 internal state).  n_inner extra refinements re-solve c with the
    updated ctilde, roughly doubling the per-sweep contraction.  K sweeps
    reach the fp16 fixed-point plateau (~6e-4 rel L2 vs the fp32 scan).
  - x-contributions of gates f,i,o are precomputed once (phase 1) and
    streamed from a DRAM scratch per sweep.

Everything on-chip is [d (partition), t (free)] so matmul outputs, the
elementwise chain, the scan, and the next sweep's matmul inputs all share
one layout; no transposes anywhere on device.
"""

import sys
import numpy as np

for _p in ("/opt/trn_rl_repo", "/root/.axon_site/_ro/trn_rl_repo"):
    if _p not in sys.path:
        sys.path.insert(0, _p)

import concourse.bass as bass
import concourse.bacc as bacc
import concourse.mybir as mybir
import concourse.tile as tile
from concourse.bass_utils import run_bass_kernel_spmd

D = 1024
T = 8192
KC = 8            # contraction chunks (1024/128)
DELTA = 128       # warm-up overlap steps
L = 1024 + DELTA  # chunk length per core
N_CORES = 8
K_SWEEPS = 14     # fixed-point sweeps
N_INNER = 2       # inner c-refinements per sweep
UNROLL = 2        # sweeps per For_i body (cross-sweep pipelining)

F16 = mybir.dt.float16
F32 = mybir.dt.float32
AF = mybir.ActivationFunctionType
ALU = mybir.AluOpType

# time-tiles per sweep: (t0, N)
M_TILES = [(0, 512), (512, 512), (1024, 128)]


def build_nc(k_sweeps: int = K_SWEEPS, n_inner: int = N_INNER, unroll: int = UNROLL):
    assert k_sweeps % unroll == 0
    nc = bacc.Bacc(None, target_bir_lowering=False, debug=False)

    # ---- I/O ----
    wh_t = nc.declare_dram_parameter("wh_t", [D, 4 * D], F16, isOutput=False)
    wc_t = nc.declare_dram_parameter("wc_t", [D, D], F16, isOutput=False)
    wx_t = nc.declare_dram_parameter("wx_t", [D, 3 * D], F16, isOutput=False)
    x_t = nc.declare_dram_parameter("x_t", [D, L], F16, isOutput=False)
    bias_f = nc.declare_dram_parameter("bias_f", [128, KC], F32, isOutput=False)
    bias_i = nc.declare_dram_parameter("bias_i", [128, KC], F32, isOutput=False)
    bias_o = nc.declare_dram_parameter("bias_o", [128, KC], F32, isOutput=False)
    bias_c = nc.declare_dram_parameter("bias_c", [128, KC], F32, isOutput=False)
    hb = nc.declare_dram_parameter("hb", [128, KC, 1], F16, isOutput=False)
    cb = nc.declare_dram_parameter("cb", [128, KC, 1], F16, isOutput=False)
    ident = nc.declare_dram_parameter("ident", [128, 128], F16, isOutput=False)
    h_out = nc.declare_dram_parameter("h_out", [128, KC, L + 1], F16, isOutput=True)

    # x-contribution of f,i,o gates, [gate*8+gd, 128, L], computed in phase 1
    pre_dram = nc.dram_tensor("pre_scratch", [24, 128, L], F16)

    with tile.TileContext(nc) as tc:
        with (
            tc.tile_pool(name="const", bufs=1) as constp,
            tc.tile_pool(name="psum", bufs=8, space="PSUM") as psum,
            tc.tile_pool(name="weights", bufs=1) as wpool,
        ):
            # constants
            bf_sb = constp.tile([128, KC], F32, tag="bf")
            bi_sb = constp.tile([128, KC], F32, tag="bi")
            bo_sb = constp.tile([128, KC], F32, tag="bo")
            bc_sb = constp.tile([128, KC], F32, tag="bc")
            id_sb = constp.tile([128, 128], F16, tag="ident")
            nc.sync.dma_start(bf_sb[:, :], bias_f[:, :])
            nc.sync.dma_start(bi_sb[:, :], bias_i[:, :])
            nc.sync.dma_start(bo_sb[:, :], bias_o[:, :])
            nc.sync.dma_start(bc_sb[:, :], bias_c[:, :])
            nc.sync.dma_start(id_sb[:, :], ident[:, :])

            # recurrent weights — loaded first so the DMA overlaps phase 1
            wh_sb = wpool.tile([128, KC, 4 * D], F16, tag="wh")
            wc_sb = wpool.tile([128, KC, D], F16, tag="wc")
            nc.sync.dma_start(
                wh_sb[:, :, :], wh_t[:, :].rearrange("(c p) m -> p c m", p=128)
            )
            nc.sync.dma_start(
                wc_sb[:, :, :], wc_t[:, :].rearrange("(c p) m -> p c m", p=128)
            )

            # ---------------- phase 1: pre = Wx @ x ----------------
            with tc.tile_pool(name="ph1", bufs=1) as ph1:
                xT_sb = ph1.tile([128, KC, L], F16, tag="xT")
                nc.sync.dma_start(
                    xT_sb[:, :, :], x_t[:, :].rearrange("(c p) t -> p c t", p=128)
                )
                for g in range(3):  # f, i, o
                    wx_sb = ph1.tile([128, KC, D], F16, tag="wxslab")
                    nc.sync.dma_start(
                        wx_sb[:, :, :],
                        wx_t[:, g * D:(g + 1) * D].rearrange(
                            "(c p) m -> p c m", p=128
                        ),
                    )
                    for (t0, N) in M_TILES:
                        for gd in range(KC):
                            ps = psum.tile([128, N], F32, tag="ps")
                            for kc in range(KC):
                                nc.tensor.matmul(
                                    ps[:, :],
                                    wx_sb[:, kc, gd * 128:(gd + 1) * 128],
                                    xT_sb[:, kc, t0:t0 + N],
                                    start=(kc == 0),
                                    stop=(kc == KC - 1),
                                )
                            pre16 = ph1.tile([128, N], F16, tag="pre16")
                            nc.vector.tensor_copy(pre16[:, :], ps[:, :])
                            nc.sync.dma_start(
                                pre_dram[g * KC + gd, :, t0:t0 + N], pre16[:, :]
                            )

            # ---------------- persistent state + sweep loop ----------------
            with (
                tc.tile_pool(name="state", bufs=1) as spool,
                tc.tile_pool(name="gates", bufs=2) as gpool,
                tc.tile_pool(name="za", bufs=1) as zpool,
                tc.tile_pool(name="work", bufs=2) as wk,
                tc.tile_pool(name="prestream", bufs=4) as prepool,
            ):
                # h/c history, col j = value at local time j-1 (col 0 = boundary)
                Hf = spool.tile([128, KC, L + 1], F16, tag="H")
                Cf = spool.tile([128, KC, L + 1], F16, tag="C")
                nc.vector.memset(Hf[:, :, :], 0.0)
                nc.vector.memset(Cf[:, :, :], 0.0)
                nc.sync.dma_start(Hf[:, :, 0:1], hb[:, :, :])
                nc.sync.dma_start(Cf[:, :, 0:1], cb[:, :, :])

                def mm_group(ps, col0, t0, N, pre_tile=None):
                    """psum <- Wh[:, col0:col0+128-block] @ H (+ I @ pre)"""
                    for kc in range(KC):
                        nc.tensor.matmul(
                            ps[:, :],
                            wh_sb[:, kc, col0:col0 + 128],
                            Hf[:, kc, t0:t0 + N],
                            start=(kc == 0),
                            stop=(kc == KC - 1) and pre_tile is None,
                        )
                    if pre_tile is not None:
                        nc.tensor.matmul(
                            ps[:, :], id_sb[:, :], pre_tile[:, :],
                            start=False, stop=True,
                        )

                def sweep_body():
                    for (t0, N) in M_TILES:
                        f_all = gpool.tile([128, KC, 512], F16, tag="f")
                        i_all = gpool.tile([128, KC, 512], F16, tag="i")
                        o_all = gpool.tile([128, KC, 512], F16, tag="o")
                        zA = zpool.tile([128, KC, 512], F32, tag="zA")

                        # prefetch pre tiles for this m-tile
                        pf_t, pi_t, po_t = [], [], []
                        for gd in range(KC):
                            pf = prepool.tile([128, N], F16, tag="pf")
                            nc.sync.dma_start(pf[:, :], pre_dram[gd, :, t0:t0 + N])
                            pf_t.append(pf)
                        for gd in range(KC):
                            pi = prepool.tile([128, N], F16, tag="pi")
                            nc.sync.dma_start(pi[:, :], pre_dram[KC + gd, :, t0:t0 + N])
                            pi_t.append(pi)
                        for gd in range(KC):
                            po = prepool.tile([128, N], F16, tag="po")
                            nc.sync.dma_start(po[:, :], pre_dram[2 * KC + gd, :, t0:t0 + N])
                            po_t.append(po)

                        # ---- ctilde pre-activation (h-part kept in zA) ----
                        ct_tiles = []
                        for ch in range(KC):
                            psA = psum.tile([128, N], F32, tag="ps")
                            mm_group(psA, 3 * D + ch * 128, t0, N)
                            nc.vector.tensor_copy(zA[:, ch, :N], psA[:, :])
                            psB = psum.tile([128, N], F32, tag="ps")
                            for kc in range(KC):
                                nc.tensor.matmul(
                                    psB[:, :],
                                    wc_sb[:, kc, ch * 128:(ch + 1) * 128],
                                    Cf[:, kc, t0:t0 + N],
                                    start=(kc == 0),
                                    stop=(kc == KC - 1),
                                )
                            tmp = wk.tile([128, N], F32, tag="tmp")
                            nc.vector.tensor_add(tmp[:, :], psB[:, :], zA[:, ch, :N])
                            ct = wk.tile([128, N], F16, tag="ct")
                            nc.scalar.activation(
                                ct[:, :], tmp[:, :], AF.Tanh, bias=bc_sb[:, ch:ch + 1]
                            )
                            ct_tiles.append(ct)

                        # ---- f and i gates ----
                        for gate_idx, (garr, pre_tiles, bias_sb) in enumerate(
                            [(f_all, pf_t, bf_sb), (i_all, pi_t, bi_sb)]
                        ):
                            for ch in range(KC):
                                ps = psum.tile([128, N], F32, tag="ps")
                                mm_group(ps, gate_idx * D + ch * 128, t0, N,
                                         pre_tiles[ch])
                                nc.scalar.activation(
                                    garr[:, ch, :N], ps[:, :], AF.Sigmoid,
                                    bias=bias_sb[:, ch:ch + 1],
                                )

                        # ---- scan 1: exact c given gates ----
                        for ch in range(KC):
                            b1 = wk.tile([128, N], F16, tag="b1")
                            nc.vector.tensor_mul(b1[:, :], i_all[:, ch, :N], ct_tiles[ch][:, :])
                            nc.vector.tensor_tensor_scan(
                                Cf[:, ch, t0 + 1:t0 + N + 1],
                                f_all[:, ch, :N],
                                b1[:, :],
                                Cf[:, ch, t0:t0 + 1],
                                ALU.mult,
                                ALU.add,
                            )

                        # ---- o gate (keeps PE busy while scans run) ----
                        for ch in range(KC):
                            ps = psum.tile([128, N], F32, tag="ps")
                            mm_group(ps, 2 * D + ch * 128, t0, N, po_t[ch])
                            nc.scalar.activation(
                                o_all[:, ch, :N], ps[:, :], AF.Sigmoid,
                                bias=bo_sb[:, ch:ch + 1],
                            )

                        # ---- inner refinements of ctilde/c with updated c ----
                        for _ in range(n_inner):
                            ps2_tiles = []
                            for ch in range(KC):
                                ps2 = psum.tile([128, N], F32, tag="ps")
                                for kc in range(KC):
                                    nc.tensor.matmul(
                                        ps2[:, :],
                                        wc_sb[:, kc, ch * 128:(ch + 1) * 128],
                                        Cf[:, kc, t0:t0 + N],
                                        start=(kc == 0),
                                        stop=(kc == KC - 1),
                                    )
                                ps2_tiles.append(ps2)
                            for ch in range(KC):
                                tmp2 = wk.tile([128, N], F32, tag="tmp")
                                nc.vector.tensor_add(tmp2[:, :], ps2_tiles[ch][:, :], zA[:, ch, :N])
                                ct2 = wk.tile([128, N], F16, tag="ct")
                                nc.scalar.activation(
                                    ct2[:, :], tmp2[:, :], AF.Tanh, bias=bc_sb[:, ch:ch + 1]
                                )
                                b2 = wk.tile([128, N], F16, tag="b1")
                                nc.vector.tensor_mul(b2[:, :], i_all[:, ch, :N], ct2[:, :])
                                nc.vector.tensor_tensor_scan(
                                    Cf[:, ch, t0 + 1:t0 + N + 1],
                                    f_all[:, ch, :N],
                                    b2[:, :],
                                    Cf[:, ch, t0:t0 + 1],
                                    ALU.mult,
                                    ALU.add,
                                )

                        # ---- h = o * tanh(c) ----
                        for ch in range(KC):
                            tch = wk.tile([128, N], F16, tag="tch")
                            nc.scalar.activation(
                                tch[:, :], Cf[:, ch, t0 + 1:t0 + N + 1], AF.Tanh
                            )
                            nc.vector.tensor_mul(
                                Hf[:, ch, t0 + 1:t0 + N + 1], o_all[:, ch, :N], tch[:, :]
                            )

                with tc.For_i(0, k_sweeps // unroll, 1):
                    for _ in range(unroll):
                        sweep_body()

                # ---------------- output ----------------
                nc.sync.dma_start(h_out[:, :, :], Hf[:, :, :])

    nc.compile()
    return nc


# ------------------------- host side -------------------------

def _prep_core_inputs(inputs):
    """Build the 8 per-core input maps from the full problem inputs."""
    x = np.asarray(inputs["target_seq"], np.float32)
    W_f = np.asarray(inputs["W_f"], np.float32)
    W_i = np.asarray(inputs["W_i"], np.float32)
    W_C = np.asarray(inputs["W_C"], np.float32)
    W_o = np.asarray(inputs["W_o"], np.float32)

    wh_t = np.concatenate(
        [W_f[:, :D].T, W_i[:, :D].T, W_o[:, :D].T, W_C[:, :D].T], axis=1
    ).astype(np.float16)                      # [D, 4D], cols = [f|i|o|C]
    wc_t = np.ascontiguousarray(W_C[:, D:].T).astype(np.float16)   # [D, D]
    wx_t = np.concatenate(
        [W_f[:, D:].T, W_i[:, D:].T, W_o[:, D:].T], axis=1
    ).astype(np.float16)                      # [D, 3D]

    def vec_pc(v):  # [D] -> [128, 8] with d = ch*128 + p
        return np.ascontiguousarray(np.asarray(v, np.float32).reshape(KC, 128).T)

    bias_f = vec_pc(inputs["b_f"])
    bias_i = vec_pc(inputs["b_i"])
    bias_o = vec_pc(inputs["b_o"])
    bias_c = vec_pc(inputs["b_C"])
    ident = np.eye(128, dtype=np.float16)

    h0 = np.asarray(inputs["encoder_h"], np.float32)
    c0 = np.asarray(inputs["encoder_c"], np.float32)

    in_maps = []
    for core in range(N_CORES):
        if core == 0:
            rows = slice(0, L)
            hbv = vec_pc(h0).astype(np.float16)[:, :, None]
            cbv = vec_pc(c0).astype(np.float16)[:, :, None]
        else:
            rows = slice(1024 * core - DELTA, 1024 * core + 1024)
            hbv = np.zeros((128, KC, 1), np.float16)
            cbv = np.zeros((128, KC, 1), np.float16)
        x_chunk_t = np.ascontiguousarray(x[rows].T).astype(np.float16)  # [D, L]
        in_maps.append({
            "wh_t": wh_t, "wc_t": wc_t, "wx_t": wx_t,
            "x_t": x_chunk_t,
            "bias_f": bias_f, "bias_i": bias_i, "bias_o": bias_o, "bias_c": bias_c,
            "hb": hbv, "cb": cbv, "ident": ident,
        })
    return in_maps


def _gather_output(results):
    """Assemble [T, D] fp32 from per-core h_out [128, 8, L+1] fp16."""
    out = np.empty((T, D), np.float32)
    for core in range(N_CORES):
        h = np.asarray(results[core]["h_out"]).reshape(128, KC, L + 1)
        # col j = h at local time j-1 ; d = ch*128 + p
        chunk = np.transpose(h, (2, 1, 0)).reshape(L + 1, D).astype(np.float32)
        if core == 0:
            out[0:1024] = chunk[1:1025]
        else:
            out[1024 * core:1024 * (core + 1)] = chunk[DELTA + 1:L + 1]
    return out


_NC_CACHE = {}


def _get_nc(k_sweeps=K_SWEEPS, n_inner=N_INNER, unroll=UNROLL):
    key = (k_sweeps, n_inner, unroll)
    if key not in _NC_CACHE:
        _NC_CACHE[key] = build_nc(k_sweeps, n_inner, unroll)
    return _NC_CACHE[key]


def kernel(**inputs) -> np.ndarray:
    nc = _get_nc()
    in_maps = _prep_core_inputs(inputs)
    res = run_bass_kernel_spmd(nc, in_maps, list(range(N_CORES)))
    return _gather_output(res.results)


if __name__ == "__main__":
    nc = build_nc()
    print("built ok")


# revision 12
# speedup vs baseline: 1.1424x; 1.0822x over previous
"""Trainium2 Bass kernel for the nn_Decoder LSTM problem.

Teacher-forced LSTM decoder, T=8192 steps, D=1024, with the quirk that the
candidate-cell gate reads [h, c] instead of [h, x].

Strategy
--------
The sequential recurrence is solved by a scan-accelerated fixed-point
(Picard/Gauss-Seidel) iteration instead of stepping 8192 times:

  - Shard time across 8 cores: core k owns output rows [1024k, 1024k+1024).
    Each core processes a chunk of L=1152 steps (128 warm-up steps with a
    zero boundary state; the LSTM's fading memory makes the boundary error
    decay below 1e-6 within ~128 steps).  Zero cross-core communication.
  - Per sweep, gate pre-activations for all timesteps in the chunk are big
    dense matmuls against the previous iterate of (h, c) in [d, t] layout.
  - Given the gates, the c-recurrence c_t = f_t*c_{t-1} + i_t*ctilde_t is
    linear-diagonal and solved EXACTLY with the DVE tensor_tensor_scan op
    (fp32 internal state).  n_inner extra refinements re-solve c with the
    updated ctilde, roughly doubling the per-sweep contraction.  K sweeps
    reach the fp16 fixed-point plateau (~6e-4 rel L2 vs the fp32 scan).
  - x-contributions of gates f,i,o are precomputed once (phase 1) and
    streamed from a DRAM scratch per sweep.

Everything on-chip is [d (partition), t (free)] so matmul outputs, the
elementwise chain, the scan, and the next sweep's matmul inputs all share
one layout; no transposes anywhere on device.
"""

import sys
import numpy as np

for _p in ("/opt/trn_rl_repo", "/root/.axon_site/_ro/trn_rl_repo"):
    if _p not in sys.path:
        sys.path.insert(0, _p)

import concourse.bass as bass
import concourse.bacc as bacc
import concourse.mybir as mybir
import concourse.tile as tile
from concourse.bass_utils import run_bass_kernel_spmd

D = 1024
T = 8192
KC = 8            # contraction chunks (1024/128)
DELTA = 128       # warm-up overlap steps
L = 1024 + DELTA  # chunk length per core
N_CORES = 8
K_SWEEPS = 14     # fixed-point sweeps
N_INNER = 2       # inner c-refinements per sweep
UNROLL = 2        # sweeps per For_i body (cross-sweep pipelining)

F16 = mybir.dt.float16
F32 = mybir.dt.float32
AF = mybir.ActivationFunctionType
ALU = mybir.AluOpType

# time-tiles per sweep: (t0, N)
M_TILES = [(0, 512), (512, 512), (1024, 128)]


def build_nc(k_sweeps: int = K_SWEEPS, n_inner: int = N_INNER, unroll: int = UNROLL):
    assert k_sweeps % unroll == 0
    nc = bacc.Bacc(None, target_bir_lowering=False, debug=False)

    # ---- I/O ----
    wh_t = nc.declare_dram_parameter("wh_t", [D, 4 * D], F16, isOutput=False)
    wc_t = nc.declare_dram_parameter("wc_t", [D, D], F16, isOutput=False)
    wx_t = nc.declare_dram_parameter("wx_t", [D, 3 * D], F16, isOutput=False)
    x_t = nc.declare_dram_parameter("x_t", [D, L], F16, isOutput=False)
    bias_f = nc.declare_dram_parameter("bias_f", [128, KC], F32, isOutput=False)
    bias_i = nc.declare_dram_parameter("bias_i", [128, KC], F32, isOutput=False)
    bias_o = nc.declare_dram_parameter("bias_o", [128, KC], F32, isOutput=False)
    bias_c = nc.declare_dram_parameter("bias_c", [128, KC], F32, isOutput=False)
    hb = nc.declare_dram_parameter("hb", [128, KC, 1], F16, isOutput=False)
    cb = nc.declare_dram_parameter("cb", [128, KC, 1], F16, isOutput=False)
    ident = nc.declare_dram_parameter("ident", [128, 128], F16, isOutput=False)
    h_out = nc.declare_dram_parameter("h_out", [128, KC, L + 1], F16, isOutput=True)

    # x-contribution of f,i,o gates, [gate*8+gd, 128, L], computed in phase 1
    pre_dram = nc.dram_tensor("pre_scratch", [24, 128, L], F16)

    with tile.TileContext(nc) as tc:
        with (
            tc.tile_pool(name="const", bufs=1) as constp,
            tc.tile_pool(name="psum", bufs=8, space="PSUM") as psum,
            tc.tile_pool(name="weights", bufs=1) as wpool,
        ):
            # constants
            bf_sb = constp.tile([128, KC], F32, tag="bf")
            bi_sb = constp.tile([128, KC], F32, tag="bi")
            bo_sb = constp.tile([128, KC], F32, tag="bo")
            bc_sb = constp.tile([128, KC], F32, tag="bc")
            id_sb = constp.tile([128, 128], F16, tag="ident")
            nc.sync.dma_start(bf_sb[:, :], bias_f[:, :])
            nc.sync.dma_start(bi_sb[:, :], bias_i[:, :])
            nc.sync.dma_start(bo_sb[:, :], bias_o[:, :])
            nc.sync.dma_start(bc_sb[:, :], bias_c[:, :])
            nc.sync.dma_start(id_sb[:, :], ident[:, :])

            # recurrent weights — loaded first so the DMA overlaps phase 1
            wh_sb = wpool.tile([128, KC, 4 * D], F16, tag="wh")
            wc_sb = wpool.tile([128, KC, D], F16, tag="wc")
            nc.sync.dma_start(
                wh_sb[:, :, :], wh_t[:, :].rearrange("(c p) m -> p c m", p=128)
            )
            nc.sync.dma_start(
                wc_sb[:, :, :], wc_t[:, :].rearrange("(c p) m -> p c m", p=128)
            )

            # ---------------- phase 1: pre = Wx @ x ----------------
            with tc.tile_pool(name="ph1", bufs=1) as ph1:
                xT_sb = ph1.tile([128, KC, L], F16, tag="xT")
                nc.sync.dma_start(
                    xT_sb[:, :, :], x_t[:, :].rearrange("(c p) t -> p c t", p=128)
                )
                for g in range(3):  # f, i, o
                    wx_sb = ph1.tile([128, KC, D], F16, tag="wxslab")
                    nc.sync.dma_start(
                        wx_sb[:, :, :],
                        wx_t[:, g * D:(g + 1) * D].rearrange(
                            "(c p) m -> p c m", p=128
                        ),
                    )
                    for (t0, N) in M_TILES:
                        for gd in range(KC):
                            ps = psum.tile([128, N], F32, tag="ps")
                            for kc in range(KC):
                                nc.tensor.matmul(
                                    ps[:, :],
                                    wx_sb[:, kc, gd * 128:(gd + 1) * 128],
                                    xT_sb[:, kc, t0:t0 + N],
                                    start=(kc == 0),
                                    stop=(kc == KC - 1),
                                )
                            pre16 = ph1.tile([128, N], F16, tag="pre16")
                            nc.vector.tensor_copy(pre16[:, :], ps[:, :])
                            nc.sync.dma_start(
                                pre_dram[g * KC + gd, :, t0:t0 + N], pre16[:, :]
                            )

            # ---------------- persistent state + sweep loop ----------------
            with (
                tc.tile_pool(name="state", bufs=1) as spool,
                tc.tile_pool(name="gates", bufs=2) as gpool,
                tc.tile_pool(name="za", bufs=1) as zpool,
                tc.tile_pool(name="work", bufs=2) as wk,
                tc.tile_pool(name="prestream", bufs=4) as prepool,
            ):
                # h/c history, col j = value at local time j-1 (col 0 = boundary)
                Hf = spool.tile([128, KC, L + 1], F16, tag="H")
                Cf = spool.tile([128, KC, L + 1], F16, tag="C")
                nc.vector.memset(Hf[:, :, :], 0.0)
                nc.vector.memset(Cf[:, :, :], 0.0)
                nc.sync.dma_start(Hf[:, :, 0:1], hb[:, :, :])
                nc.sync.dma_start(Cf[:, :, 0:1], cb[:, :, :])

                def mm_group(ps, col0, t0, N, pre_tile=None):
                    """psum <- Wh[:, col0:col0+128-block] @ H (+ I @ pre)"""
                    for kc in range(KC):
                        nc.tensor.matmul(
                            ps[:, :],
                            wh_sb[:, kc, col0:col0 + 128],
                            Hf[:, kc, t0:t0 + N],
                            start=(kc == 0),
                            stop=(kc == KC - 1) and pre_tile is None,
                        )
                    if pre_tile is not None:
                        nc.tensor.matmul(
                            ps[:, :], id_sb[:, :], pre_tile[:, :],
                            start=False, stop=True,
                        )

                def sweep_body(first=False):
                    """first=True: H=C=0 everywhere (except boundary col), so all
                    h/c matmuls vanish — f/i/o come straight from pre via ACT and
                    ctilde's h-part is zero.  Numerically identical to the full
                    sweep on zero state; skips ~70% of the sweep's matmuls and
                    hides the weight-load DMA."""
                    for (t0, N) in M_TILES:
                        f_all = gpool.tile([128, KC, 512], F16, tag="f")
                        i_all = gpool.tile([128, KC, 512], F16, tag="i")
                        o_all = gpool.tile([128, KC, 512], F16, tag="o")
                        zA = zpool.tile([128, KC, 512], F32, tag="zA")

                        # prefetch pre tiles for this m-tile
                        pf_t, pi_t, po_t = [], [], []
                        for gd in range(KC):
                            pf = prepool.tile([128, N], F16, tag="pf")
                            nc.sync.dma_start(pf[:, :], pre_dram[gd, :, t0:t0 + N])
                            pf_t.append(pf)
                        for gd in range(KC):
                            pi = prepool.tile([128, N], F16, tag="pi")
                            nc.sync.dma_start(pi[:, :], pre_dram[KC + gd, :, t0:t0 + N])
                            pi_t.append(pi)
                        for gd in range(KC):
                            po = prepool.tile([128, N], F16, tag="po")
                            nc.sync.dma_start(po[:, :], pre_dram[2 * KC + gd, :, t0:t0 + N])
                            po_t.append(po)

                        # ---- ctilde pre-activation (h-part kept in zA) ----
                        ct_tiles = []
                        if first:
                            nc.vector.memset(zA[:, :, :], 0.0)
                        for ch in range(KC):
                            if first:
                                # H = C = 0: ctilde = tanh(b_C)
                                ct = wk.tile([128, N], F16, tag="ct")
                                nc.scalar.activation(
                                    ct[:, :], pf_t[ch][:, :], AF.Tanh,
                                    bias=bc_sb[:, ch:ch + 1], scale=0.0,
                                )
                                ct_tiles.append(ct)
                                continue
                            psA = psum.tile([128, N], F32, tag="ps")
                            mm_group(psA, 3 * D + ch * 128, t0, N)
                            nc.vector.tensor_copy(zA[:, ch, :N], psA[:, :])
                            psB = psum.tile([128, N], F32, tag="ps")
                            for kc in range(KC):
                                nc.tensor.matmul(
                                    psB[:, :],
                                    wc_sb[:, kc, ch * 128:(ch + 1) * 128],
                                    Cf[:, kc, t0:t0 + N],
                                    start=(kc == 0),
                                    stop=(kc == KC - 1),
                                )
                            tmp = wk.tile([128, N], F32, tag="tmp")
                            nc.vector.tensor_add(tmp[:, :], psB[:, :], zA[:, ch, :N])
                            ct = wk.tile([128, N], F16, tag="ct")
                            nc.scalar.activation(
                                ct[:, :], tmp[:, :], AF.Tanh, bias=bc_sb[:, ch:ch + 1]
                            )
                            ct_tiles.append(ct)

                        # ---- f and i gates ----
                        for gate_idx, (garr, pre_tiles, bias_sb) in enumerate(
                            [(f_all, pf_t, bf_sb), (i_all, pi_t, bi_sb)]
                        ):
                            for ch in range(KC):
                                if first:
                                    nc.scalar.activation(
                                        garr[:, ch, :N], pre_tiles[ch][:, :],
                                        AF.Sigmoid, bias=bias_sb[:, ch:ch + 1],
                                    )
                                    continue
                                ps = psum.tile([128, N], F32, tag="ps")
                                mm_group(ps, gate_idx * D + ch * 128, t0, N,
                                         pre_tiles[ch])
                                nc.scalar.activation(
                                    garr[:, ch, :N], ps[:, :], AF.Sigmoid,
                                    bias=bias_sb[:, ch:ch + 1],
                                )

                        # ---- scan 1: exact c given gates ----
                        for ch in range(KC):
                            b1 = wk.tile([128, N], F16, tag="b1")
                            nc.vector.tensor_mul(b1[:, :], i_all[:, ch, :N], ct_tiles[ch][:, :])
                            nc.vector.tensor_tensor_scan(
                                Cf[:, ch, t0 + 1:t0 + N + 1],
                                f_all[:, ch, :N],
                                b1[:, :],
                                Cf[:, ch, t0:t0 + 1],
                                ALU.mult,
                                ALU.add,
                            )

                        # ---- o gate (keeps PE busy while scans run) ----
                        for ch in range(KC):
                            if first:
                                nc.scalar.activation(
                                    o_all[:, ch, :N], po_t[ch][:, :], AF.Sigmoid,
                                    bias=bo_sb[:, ch:ch + 1],
                                )
                                continue
                            ps = psum.tile([128, N], F32, tag="ps")
                            mm_group(ps, 2 * D + ch * 128, t0, N, po_t[ch])
                            nc.scalar.activation(
                                o_all[:, ch, :N], ps[:, :], AF.Sigmoid,
                                bias=bo_sb[:, ch:ch + 1],
                            )

                        # ---- inner refinements of ctilde/c with updated c ----
                        for _ in range(n_inner):
                            ps2_tiles = []
                            for ch in range(KC):
                                ps2 = psum.tile([128, N], F32, tag="ps")
                                for kc in range(KC):
                                    nc.tensor.matmul(
                                        ps2[:, :],
                                        wc_sb[:, kc, ch * 128:(ch + 1) * 128],
                                        Cf[:, kc, t0:t0 + N],
                                        start=(kc == 0),
                                        stop=(kc == KC - 1),
                                    )
                                ps2_tiles.append(ps2)
                            for ch in range(KC):
                                tmp2 = wk.tile([128, N], F32, tag="tmp")
                                nc.vector.tensor_add(tmp2[:, :], ps2_tiles[ch][:, :], zA[:, ch, :N])
                                ct2 = wk.tile([128, N], F16, tag="ct")
                                nc.scalar.activation(
                                    ct2[:, :], tmp2[:, :], AF.Tanh, bias=bc_sb[:, ch:ch + 1]
                                )
                                b2 = wk.tile([128, N], F16, tag="b1")
                                nc.vector.tensor_mul(b2[:, :], i_all[:, ch, :N], ct2[:, :])
                                nc.vector.tensor_tensor_scan(
                                    Cf[:, ch, t0 + 1:t0 + N + 1],
                                    f_all[:, ch, :N],
                                    b2[:, :],
                                    Cf[:, ch, t0:t0 + 1],
                                    ALU.mult,
                                    ALU.add,
                                )

                        # ---- h = o * tanh(c) ----
                        for ch in range(KC):
                            tch = wk.tile([128, N], F16, tag="tch")
                            nc.scalar.activation(
                                tch[:, :], Cf[:, ch, t0 + 1:t0 + N + 1], AF.Tanh
                            )
                            nc.vector.tensor_mul(
                                Hf[:, ch, t0 + 1:t0 + N + 1], o_all[:, ch, :N], tch[:, :]
                            )

                # sweep 0 (zero-state shortcut) + sweep 1 inline, rest looped
                sweep_body(first=True)
                sweep_body()
                n_loop = k_sweeps - 2
                assert n_loop % unroll == 0
                with tc.For_i(
                    0, n_loop // unroll, 1,
                    hint_engines=(
                        mybir.EngineType.PE, mybir.EngineType.DVE,
                        mybir.EngineType.Activation, mybir.EngineType.SP,
                        mybir.EngineType.Pool,
                    ),
                ):
                    for _ in range(unroll):
                        sweep_body()

                # ---------------- output ----------------
                nc.sync.dma_start(h_out[:, :, :], Hf[:, :, :])

    nc.compile()
    return nc


# ------------------------- host side -------------------------

def _prep_core_inputs(inputs):
    """Build the 8 per-core input maps from the full problem inputs."""
    x = np.asarray(inputs["target_seq"], np.float32)
    W_f = np.asarray(inputs["W_f"], np.float32)
    W_i = np.asarray(inputs["W_i"], np.float32)
    W_C = np.asarray(inputs["W_C"], np.float32)
    W_o = np.asarray(inputs["W_o"], np.float32)

    wh_t = np.concatenate(
        [W_f[:, :D].T, W_i[:, :D].T, W_o[:, :D].T, W_C[:, :D].T], axis=1
    ).astype(np.float16)                      # [D, 4D], cols = [f|i|o|C]
    wc_t = np.ascontiguousarray(W_C[:, D:].T).astype(np.float16)   # [D, D]
    wx_t = np.concatenate(
        [W_f[:, D:].T, W_i[:, D:].T, W_o[:, D:].T], axis=1
    ).astype(np.float16)                      # [D, 3D]

    def vec_pc(v):  # [D] -> [128, 8] with d = ch*128 + p
        return np.ascontiguousarray(np.asarray(v, np.float32).reshape(KC, 128).T)

    bias_f = vec_pc(inputs["b_f"])
    bias_i = vec_pc(inputs["b_i"])
    bias_o = vec_pc(inputs["b_o"])
    bias_c = vec_pc(inputs["b_C"])
    ident = np.eye(128, dtype=np.float16)

    h0 = np.asarray(inputs["encoder_h"], np.float32)
    c0 = np.asarray(inputs["encoder_c"], np.float32)

    in_maps = []
    for core in range(N_CORES):
        if core == 0:
            rows = slice(0, L)
            hbv = vec_pc(h0).astype(np.float16)[:, :, None]
            cbv = vec_pc(c0).astype(np.float16)[:, :, None]
        else:
            rows = slice(1024 * core - DELTA, 1024 * core + 1024)
            hbv = np.zeros((128, KC, 1), np.float16)
            cbv = np.zeros((128, KC, 1), np.float16)
        x_chunk_t = np.ascontiguousarray(x[rows].T).astype(np.float16)  # [D, L]
        in_maps.append({
            "wh_t": wh_t, "wc_t": wc_t, "wx_t": wx_t,
            "x_t": x_chunk_t,
            "bias_f": bias_f, "bias_i": bias_i, "bias_o": bias_o, "bias_c": bias_c,
            "hb": hbv, "cb": cbv, "ident": ident,
        })
    return in_maps


def _gather_output(results):
    """Assemble [T, D] fp32 from per-core h_out [128, 8, L+1] fp16."""
    out = np.empty((T, D), np.float32)
    for core in range(N_CORES):
        h = np.asarray(results[core]["h_out"]).reshape(128, KC, L + 1)
        # col j = h at local time j-1 ; d = ch*128 + p
        chunk = np.transpose(h, (2, 1, 0)).reshape(L + 1, D).astype(np.float32)
        if core == 0:
            out[0:1024] = chunk[1:1025]
        else:
            out[1024 * core:1024 * (core + 1)] = chunk[DELTA + 1:L + 1]
    return out


_NC_CACHE = {}


def _get_nc(k_sweeps=K_SWEEPS, n_inner=N_INNER, unroll=UNROLL):
    key = (k_sweeps, n_inner, unroll)
    if key not in _NC_CACHE:
        _NC_CACHE[key] = build_nc(k_sweeps, n_inner, unroll)
    return _NC_CACHE[key]


def kernel(**inputs) -> np.ndarray:
    nc = _get_nc()
    in_maps = _prep_core_inputs(inputs)
    res = run_bass_kernel_spmd(nc, in_maps, list(range(N_CORES)))
    return _gather_output(res.results)


if __name__ == "__main__":
    nc = build_nc()
    print("built ok")


# revision 17
# speedup vs baseline: 1.1490x; 1.0058x over previous
"""Trainium2 Bass kernel for the nn_Decoder LSTM problem.

Teacher-forced LSTM decoder, T=8192 steps, D=1024, with the quirk that the
candidate-cell gate reads [h, c] instead of [h, x].

Strategy
--------
The sequential recurrence is solved by a scan-accelerated fixed-point
(Picard/Gauss-Seidel) iteration instead of stepping 8192 times:

  - Shard time across 8 cores: core k owns output rows [1024k, 1024k+1024).
    Each core processes a chunk of L=1152 steps (128 warm-up steps with a
    zero boundary state; the LSTM's fading memory makes the boundary error
    decay below 1e-6 within ~128 steps).  Zero cross-core communication.
  - Per sweep, gate pre-activations for all timesteps in the chunk are big
    dense matmuls against the previous iterate of (h, c) in [d, t] layout.
  - Given the gates, the c-recurrence c_t = f_t*c_{t-1} + i_t*ctilde_t is
    linear-diagonal and solved EXACTLY with the DVE tensor_tensor_scan op
    (fp32 internal state).  n_inner extra refinements re-solve c with the
    updated ctilde, roughly doubling the per-sweep contraction.  K sweeps
    reach the fp16 fixed-point plateau (~6e-4 rel L2 vs the fp32 scan).
  - x-contributions of gates f,i,o are precomputed once (phase 1) and
    streamed from a DRAM scratch per sweep.

Everything on-chip is [d (partition), t (free)] so matmul outputs, the
elementwise chain, the scan, and the next sweep's matmul inputs all share
one layout; no transposes anywhere on device.
"""

import sys
import numpy as np

for _p in ("/opt/trn_rl_repo", "/root/.axon_site/_ro/trn_rl_repo"):
    if _p not in sys.path:
        sys.path.insert(0, _p)

import concourse.bass as bass
import concourse.bacc as bacc
import concourse.mybir as mybir
import concourse.tile as tile
from concourse.bass_utils import run_bass_kernel_spmd

D = 1024
T = 8192
KC = 8            # contraction chunks (1024/128)
DELTA = 128       # warm-up overlap steps
L = 1024 + DELTA  # chunk length per core
N_CORES = 8
K_SWEEPS = 14     # fixed-point sweeps
N_INNER = 2       # inner c-refinements per sweep
UNROLL = 2        # sweeps per For_i body (cross-sweep pipelining)

F16 = mybir.dt.float16
F32 = mybir.dt.float32
AF = mybir.ActivationFunctionType
ALU = mybir.AluOpType

# time-tiles per sweep: (t0, N)
M_TILES = [(0, 512), (512, 512), (1024, 128)]


def build_nc(k_sweeps: int = K_SWEEPS, n_inner: int = N_INNER, unroll: int = UNROLL):
    assert k_sweeps % unroll == 0
    nc = bacc.Bacc(None, target_bir_lowering=False, debug=False)

    # ---- I/O ----
    wh_t = nc.declare_dram_parameter("wh_t", [D, 4 * D], F16, isOutput=False)
    wc_t = nc.declare_dram_parameter("wc_t", [D, D], F16, isOutput=False)
    wx_t = nc.declare_dram_parameter("wx_t", [D, 3 * D], F16, isOutput=False)
    x_t = nc.declare_dram_parameter("x_t", [D, L], F16, isOutput=False)
    bias_f = nc.declare_dram_parameter("bias_f", [128, KC], F32, isOutput=False)
    bias_i = nc.declare_dram_parameter("bias_i", [128, KC], F32, isOutput=False)
    bias_o = nc.declare_dram_parameter("bias_o", [128, KC], F32, isOutput=False)
    bias_c = nc.declare_dram_parameter("bias_c", [128, KC], F32, isOutput=False)
    hb = nc.declare_dram_parameter("hb", [128, KC, 1], F16, isOutput=False)
    cb = nc.declare_dram_parameter("cb", [128, KC, 1], F16, isOutput=False)
    ident = nc.declare_dram_parameter("ident", [128, 128], F16, isOutput=False)
    h_out = nc.declare_dram_parameter("h_out", [128, KC, L + 1], F16, isOutput=True)

    # x-contribution of f,i,o gates, [gate*8+gd, 128, L], computed in phase 1
    pre_dram = nc.dram_tensor("pre_scratch", [24, 128, L], F16)

    with tile.TileContext(nc) as tc:
        with (
            tc.tile_pool(name="const", bufs=1) as constp,
            tc.tile_pool(name="psum", bufs=8, space="PSUM") as psum,
            tc.tile_pool(name="weights", bufs=1) as wpool,
        ):
            # constants
            bf_sb = constp.tile([128, KC], F32, tag="bf")
            bi_sb = constp.tile([128, KC], F32, tag="bi")
            bo_sb = constp.tile([128, KC], F32, tag="bo")
            bc_sb = constp.tile([128, KC], F32, tag="bc")
            id_sb = constp.tile([128, 128], F16, tag="ident")
            nc.sync.dma_start(bf_sb[:, :], bias_f[:, :])
            nc.sync.dma_start(bi_sb[:, :], bias_i[:, :])
            nc.sync.dma_start(bo_sb[:, :], bias_o[:, :])
            nc.sync.dma_start(bc_sb[:, :], bias_c[:, :])
            nc.sync.dma_start(id_sb[:, :], ident[:, :])

            # recurrent weights — loaded first so the DMA overlaps phase 1
            wh_sb = wpool.tile([128, KC, 4 * D], F16, tag="wh")
            wc_sb = wpool.tile([128, KC, D], F16, tag="wc")
            nc.sync.dma_start(
                wh_sb[:, :, :], wh_t[:, :].rearrange("(c p) m -> p c m", p=128)
            )
            nc.sync.dma_start(
                wc_sb[:, :, :], wc_t[:, :].rearrange("(c p) m -> p c m", p=128)
            )

            # ---------------- phase 1: pre = Wx @ x ----------------
            with tc.tile_pool(name="ph1", bufs=1) as ph1:
                xT_sb = ph1.tile([128, KC, L], F16, tag="xT")
                nc.sync.dma_start(
                    xT_sb[:, :, :], x_t[:, :].rearrange("(c p) t -> p c t", p=128)
                )
                for g in range(3):  # f, i, o
                    wx_sb = ph1.tile([128, KC, D], F16, tag="wxslab")
                    nc.sync.dma_start(
                        wx_sb[:, :, :],
                        wx_t[:, g * D:(g + 1) * D].rearrange(
                            "(c p) m -> p c m", p=128
                        ),
                    )
                    for (t0, N) in M_TILES:
                        for gd in range(KC):
                            ps = psum.tile([128, N], F32, tag="ps")
                            for kc in range(KC):
                                nc.tensor.matmul(
                                    ps[:, :],
                                    wx_sb[:, kc, gd * 128:(gd + 1) * 128],
                                    xT_sb[:, kc, t0:t0 + N],
                                    start=(kc == 0),
                                    stop=(kc == KC - 1),
                                )
                            pre16 = ph1.tile([128, N], F16, tag="pre16")
                            nc.vector.tensor_copy(pre16[:, :], ps[:, :])
                            nc.sync.dma_start(
                                pre_dram[g * KC + gd, :, t0:t0 + N], pre16[:, :]
                            )

            # ---------------- persistent state + sweep loop ----------------
            with (
                tc.tile_pool(name="state", bufs=1) as spool,
                tc.tile_pool(name="gates", bufs=2) as gpool,
                tc.tile_pool(name="za", bufs=1) as zpool,
                tc.tile_pool(name="work", bufs=2) as wk,
                tc.tile_pool(name="prestream", bufs=4) as prepool,
            ):
                # h/c history, col j = value at local time j-1 (col 0 = boundary)
                Hf = spool.tile([128, KC, L + 1], F16, tag="H")
                Cf = spool.tile([128, KC, L + 1], F16, tag="C")
                nc.vector.memset(Hf[:, :, :], 0.0)
                nc.vector.memset(Cf[:, :, :], 0.0)
                nc.sync.dma_start(Hf[:, :, 0:1], hb[:, :, :])
                nc.sync.dma_start(Cf[:, :, 0:1], cb[:, :, :])

                def mm_group(ps, col0, t0, N, pre_tile=None):
                    """psum <- Wh[:, col0:col0+128-block] @ H (+ I @ pre)"""
                    for kc in range(KC):
                        nc.tensor.matmul(
                            ps[:, :],
                            wh_sb[:, kc, col0:col0 + 128],
                            Hf[:, kc, t0:t0 + N],
                            start=(kc == 0),
                            stop=(kc == KC - 1) and pre_tile is None,
                        )
                    if pre_tile is not None:
                        nc.tensor.matmul(
                            ps[:, :], id_sb[:, :], pre_tile[:, :],
                            start=False, stop=True,
                        )

                def sweep_body(first=False):
                    """first=True: H=C=0, so all h/c matmuls vanish — f/i/o come
                    straight from pre via ACT; skips ~70% of the sweep's matmuls
                    and hides the weight-load DMA.

                    Boundary-column deferral: each m-tile's LAST output column
                    (h/c at time t0+N-1, i.e. col t0+N) is the only value the
                    next m-tile's matmuls overlap with.  The scans/P3 write only
                    N-1 columns; the last column is emitted in the NEXT m-tile's
                    section after its matmul groups, which therefore read the
                    one-sweep-stale value (identical at the fixed point) and no
                    longer wait on this m-tile's elementwise tail.  The next
                    m-tile's scan initial and inner refinements still read the
                    fresh value (emitted before them)."""
                    deferred = None
                    for (t0, N) in M_TILES:
                        f_all = gpool.tile([128, KC, 512], F16, tag="f")
                        i_all = gpool.tile([128, KC, 512], F16, tag="i")
                        o_all = gpool.tile([128, KC, 512], F16, tag="o")
                        zA = zpool.tile([128, KC, 512], F32, tag="zA")

                        # prefetch pre tiles for this m-tile
                        pf_t, pi_t, po_t = [], [], []
                        for gd in range(KC):
                            pf = prepool.tile([128, N], F16, tag="pf")
                            nc.sync.dma_start(pf[:, :], pre_dram[gd, :, t0:t0 + N])
                            pf_t.append(pf)
                        for gd in range(KC):
                            pi = prepool.tile([128, N], F16, tag="pi")
                            nc.sync.dma_start(pi[:, :], pre_dram[KC + gd, :, t0:t0 + N])
                            pi_t.append(pi)
                        for gd in range(KC):
                            po = prepool.tile([128, N], F16, tag="po")
                            nc.sync.dma_start(po[:, :], pre_dram[2 * KC + gd, :, t0:t0 + N])
                            po_t.append(po)

                        # ---- ctilde pre-activation (h-part kept in zA) ----
                        ct_tiles = []
                        if first:
                            nc.vector.memset(zA[:, :, :], 0.0)
                        for ch in range(KC):
                            if first:
                                # H = C = 0: ctilde = tanh(b_C)
                                ct = wk.tile([128, N], F16, tag="ct")
                                nc.scalar.activation(
                                    ct[:, :], pf_t[ch][:, :], AF.Tanh,
                                    bias=bc_sb[:, ch:ch + 1], scale=0.0,
                                )
                                ct_tiles.append(ct)
                                continue
                            psA = psum.tile([128, N], F32, tag="ps")
                            mm_group(psA, 3 * D + ch * 128, t0, N)
                            nc.vector.tensor_copy(zA[:, ch, :N], psA[:, :])
                            psB = psum.tile([128, N], F32, tag="ps")
                            for kc in range(KC):
                                nc.tensor.matmul(
                                    psB[:, :],
                                    wc_sb[:, kc, ch * 128:(ch + 1) * 128],
                                    Cf[:, kc, t0:t0 + N],
                                    start=(kc == 0),
                                    stop=(kc == KC - 1),
                                )
                            tmp = wk.tile([128, N], F32, tag="tmp")
                            nc.vector.tensor_add(tmp[:, :], psB[:, :], zA[:, ch, :N])
                            ct = wk.tile([128, N], F16, tag="ct")
                            nc.scalar.activation(
                                ct[:, :], tmp[:, :], AF.Tanh, bias=bc_sb[:, ch:ch + 1]
                            )
                            ct_tiles.append(ct)

                        # ---- f and i gates ----
                        for gate_idx, (garr, pre_tiles, bias_sb) in enumerate(
                            [(f_all, pf_t, bf_sb), (i_all, pi_t, bi_sb)]
                        ):
                            for ch in range(KC):
                                if first:
                                    nc.scalar.activation(
                                        garr[:, ch, :N], pre_tiles[ch][:, :],
                                        AF.Sigmoid, bias=bias_sb[:, ch:ch + 1],
                                    )
                                    continue
                                ps = psum.tile([128, N], F32, tag="ps")
                                mm_group(ps, gate_idx * D + ch * 128, t0, N,
                                         pre_tiles[ch])
                                nc.scalar.activation(
                                    garr[:, ch, :N], ps[:, :], AF.Sigmoid,
                                    bias=bias_sb[:, ch:ch + 1],
                                )

                        # ---- flush previous m-tile's deferred boundary column
                        # (after this tile's matmul groups, before its scans) ----
                        if deferred is not None:
                            deferred()
                            deferred = None

                        # ---- scan 1: exact c given gates (N-1 cols) ----
                        for ch in range(KC):
                            b1 = wk.tile([128, N], F16, tag="b1")
                            nc.vector.tensor_mul(b1[:, :], i_all[:, ch, :N], ct_tiles[ch][:, :])
                            nc.vector.tensor_tensor_scan(
                                Cf[:, ch, t0 + 1:t0 + N],
                                f_all[:, ch, :N - 1],
                                b1[:, :N - 1],
                                Cf[:, ch, t0:t0 + 1],
                                ALU.mult,
                                ALU.add,
                            )

                        # ---- o gate (keeps PE busy while scans run) ----
                        for ch in range(KC):
                            if first:
                                nc.scalar.activation(
                                    o_all[:, ch, :N], po_t[ch][:, :], AF.Sigmoid,
                                    bias=bo_sb[:, ch:ch + 1],
                                )
                                continue
                            ps = psum.tile([128, N], F32, tag="ps")
                            mm_group(ps, 2 * D + ch * 128, t0, N, po_t[ch])
                            nc.scalar.activation(
                                o_all[:, ch, :N], ps[:, :], AF.Sigmoid,
                                bias=bo_sb[:, ch:ch + 1],
                            )

                        # ---- inner refinements of ctilde/c with updated c ----
                        blastcol = gpool.tile([128, KC], F16, tag="blc")
                        for inner_i in range(n_inner):
                            is_last = inner_i == n_inner - 1
                            ps2_tiles = []
                            for ch in range(KC):
                                ps2 = psum.tile([128, N], F32, tag="ps")
                                for kc in range(KC):
                                    nc.tensor.matmul(
                                        ps2[:, :],
                                        wc_sb[:, kc, ch * 128:(ch + 1) * 128],
                                        Cf[:, kc, t0:t0 + N],
                                        start=(kc == 0),
                                        stop=(kc == KC - 1),
                                    )
                                ps2_tiles.append(ps2)
                            for ch in range(KC):
                                tmp2 = wk.tile([128, N], F32, tag="tmp")
                                nc.vector.tensor_add(tmp2[:, :], ps2_tiles[ch][:, :], zA[:, ch, :N])
                                ct2 = wk.tile([128, N], F16, tag="ct")
                                nc.scalar.activation(
                                    ct2[:, :], tmp2[:, :], AF.Tanh, bias=bc_sb[:, ch:ch + 1]
                                )
                                b2 = wk.tile([128, N], F16, tag="b1")
                                nc.vector.tensor_mul(b2[:, :], i_all[:, ch, :N], ct2[:, :])
                                if is_last:
                                    nc.vector.tensor_copy(
                                        blastcol[:, ch:ch + 1], b2[:, N - 1:N]
                                    )
                                nc.vector.tensor_tensor_scan(
                                    Cf[:, ch, t0 + 1:t0 + N],
                                    f_all[:, ch, :N - 1],
                                    b2[:, :N - 1],
                                    Cf[:, ch, t0:t0 + 1],
                                    ALU.mult,
                                    ALU.add,
                                )

                        # ---- h = o * tanh(c) over the N-1 materialized cols ----
                        for ch in range(KC):
                            tch = wk.tile([128, N], F16, tag="tch")
                            nc.scalar.activation(
                                tch[:, :N - 1], Cf[:, ch, t0 + 1:t0 + N], AF.Tanh
                            )
                            nc.vector.tensor_mul(
                                Hf[:, ch, t0 + 1:t0 + N], o_all[:, ch, :N - 1],
                                tch[:, :N - 1],
                            )

                        # ---- deferred write of the last column (t0+N) ----
                        def make_deferred(t0=t0, N=N, f_prev=f_all, o_prev=o_all,
                                          b_prev=blastcol):
                            def flush():
                                for ch in range(KC):
                                    nc.vector.tensor_tensor_scan(
                                        Cf[:, ch, t0 + N:t0 + N + 1],
                                        f_prev[:, ch, N - 1:N],
                                        b_prev[:, ch:ch + 1],
                                        Cf[:, ch, t0 + N - 1:t0 + N],
                                        ALU.mult,
                                        ALU.add,
                                    )
                                    tc1 = wk.tile([128, 1], F16, tag="tc1")
                                    nc.scalar.activation(
                                        tc1[:, :], Cf[:, ch, t0 + N:t0 + N + 1], AF.Tanh
                                    )
                                    nc.vector.tensor_mul(
                                        Hf[:, ch, t0 + N:t0 + N + 1],
                                        o_prev[:, ch, N - 1:N], tc1[:, :],
                                    )
                            return flush

                        deferred = make_deferred()

                    # last m-tile of the sweep: flush immediately (its boundary
                    # column has no same-sweep matmul readers)
                    deferred()

                # sweep 0 (zero-state shortcut) + sweep 1 inline, rest looped
                sweep_body(first=True)
                sweep_body()
                n_loop = k_sweeps - 2
                assert n_loop % unroll == 0
                with tc.For_i(
                    0, n_loop // unroll, 1,
                    hint_engines=(
                        mybir.EngineType.PE, mybir.EngineType.DVE,
                        mybir.EngineType.Activation, mybir.EngineType.SP,
                        mybir.EngineType.Pool,
                    ),
                ):
                    for _ in range(unroll):
                        sweep_body()

                # ---------------- output ----------------
                nc.sync.dma_start(h_out[:, :, :], Hf[:, :, :])

    nc.compile()
    return nc


# ------------------------- host side -------------------------

def _prep_core_inputs(inputs):
    """Build the 8 per-core input maps from the full problem inputs."""
    x = np.asarray(inputs["target_seq"], np.float32)
    W_f = np.asarray(inputs["W_f"], np.float32)
    W_i = np.asarray(inputs["W_i"], np.float32)
    W_C = np.asarray(inputs["W_C"], np.float32)
    W_o = np.asarray(inputs["W_o"], np.float32)

    wh_t = np.concatenate(
        [W_f[:, :D].T, W_i[:, :D].T, W_o[:, :D].T, W_C[:, :D].T], axis=1
    ).astype(np.float16)                      # [D, 4D], cols = [f|i|o|C]
    wc_t = np.ascontiguousarray(W_C[:, D:].T).astype(np.float16)   # [D, D]
    wx_t = np.concatenate(
        [W_f[:, D:].T, W_i[:, D:].T, W_o[:, D:].T], axis=1
    ).astype(np.float16)                      # [D, 3D]

    def vec_pc(v):  # [D] -> [128, 8] with d = ch*128 + p
        return np.ascontiguousarray(np.asarray(v, np.float32).reshape(KC, 128).T)

    bias_f = vec_pc(inputs["b_f"])
    bias_i = vec_pc(inputs["b_i"])
    bias_o = vec_pc(inputs["b_o"])
    bias_c = vec_pc(inputs["b_C"])
    ident = np.eye(128, dtype=np.float16)

    h0 = np.asarray(inputs["encoder_h"], np.float32)
    c0 = np.asarray(inputs["encoder_c"], np.float32)

    in_maps = []
    for core in range(N_CORES):
        if core == 0:
            rows = slice(0, L)
            hbv = vec_pc(h0).astype(np.float16)[:, :, None]
            cbv = vec_pc(c0).astype(np.float16)[:, :, None]
        else:
            rows = slice(1024 * core - DELTA, 1024 * core + 1024)
            hbv = np.zeros((128, KC, 1), np.float16)
            cbv = np.zeros((128, KC, 1), np.float16)
        x_chunk_t = np.ascontiguousarray(x[rows].T).astype(np.float16)  # [D, L]
        in_maps.append({
            "wh_t": wh_t, "wc_t": wc_t, "wx_t": wx_t,
            "x_t": x_chunk_t,
            "bias_f": bias_f, "bias_i": bias_i, "bias_o": bias_o, "bias_c": bias_c,
            "hb": hbv, "cb": cbv, "ident": ident,
        })
    return in_maps


def _gather_output(results):
    """Assemble [T, D] fp32 from per-core h_out [128, 8, L+1] fp16."""
    out = np.empty((T, D), np.float32)
    for core in range(N_CORES):
        h = np.asarray(results[core]["h_out"]).reshape(128, KC, L + 1)
        # col j = h at local time j-1 ; d = ch*128 + p
        chunk = np.transpose(h, (2, 1, 0)).reshape(L + 1, D).astype(np.float32)
        if core == 0:
            out[0:1024] = chunk[1:1025]
        else:
            out[1024 * core:1024 * (core + 1)] = chunk[DELTA + 1:L + 1]
    return out


_NC_CACHE = {}


def _get_nc(k_sweeps=K_SWEEPS, n_inner=N_INNER, unroll=UNROLL):
    key = (k_sweeps, n_inner, unroll)
    if key not in _NC_CACHE:
        _NC_CACHE[key] = build_nc(k_sweeps, n_inner, unroll)
    return _NC_CACHE[key]


def kernel(**inputs) -> np.ndarray:
    nc = _get_nc()
    in_maps = _prep_core_inputs(inputs)
    res = run_bass_kernel_spmd(nc, in_maps, list(range(N_CORES)))
    return _gather_output(res.results)


if __name__ == "__main__":
    nc = build_nc()
    print("built ok")


# revision 18
# speedup vs baseline: 1.2501x; 1.0880x over previous
"""Trainium2 Bass kernel for the nn_Decoder LSTM problem.

Teacher-forced LSTM decoder, T=8192 steps, D=1024, with the quirk that the
candidate-cell gate reads [h, c] instead of [h, x].

Strategy
--------
The sequential recurrence is solved by a scan-accelerated fixed-point
(Picard/Gauss-Seidel) iteration instead of stepping 8192 times:

  - Shard time across 8 cores: core k owns output rows [1024k, 1024k+1024).
    Each core processes a chunk of L=1152 steps (128 warm-up steps with a
    zero boundary state; the LSTM's fading memory makes the boundary error
    decay below 1e-6 within ~128 steps).  Zero cross-core communication.
  - Per sweep, gate pre-activations for all timesteps in the chunk are big
    dense matmuls against the previous iterate of (h, c) in [d, t] layout.
  - Given the gates, the c-recurrence c_t = f_t*c_{t-1} + i_t*ctilde_t is
    linear-diagonal and solved EXACTLY with the DVE tensor_tensor_scan op
    (fp32 internal state).  n_inner extra refinements re-solve c with the
    updated ctilde, roughly doubling the per-sweep contraction.  K sweeps
    reach the fp16 fixed-point plateau (~6e-4 rel L2 vs the fp32 scan).
  - x-contributions of gates f,i,o are precomputed once (phase 1) and
    streamed from a DRAM scratch per sweep.

Everything on-chip is [d (partition), t (free)] so matmul outputs, the
elementwise chain, the scan, and the next sweep's matmul inputs all share
one layout; no transposes anywhere on device.
"""

import sys
import numpy as np

for _p in ("/opt/trn_rl_repo", "/root/.axon_site/_ro/trn_rl_repo"):
    if _p not in sys.path:
        sys.path.insert(0, _p)

import concourse.bass as bass
import concourse.bacc as bacc
import concourse.mybir as mybir
import concourse.tile as tile
from concourse.bass_utils import run_bass_kernel_spmd

D = 1024
T = 8192
KC = 8            # contraction chunks (1024/128)
DELTA = 64        # warm-up overlap steps
L = 1024 + DELTA  # chunk length per core
N_CORES = 8
K_SWEEPS = 14     # fixed-point sweeps
N_INNER = 2       # inner c-refinements per sweep
UNROLL = 2        # sweeps per For_i body (cross-sweep pipelining)

F16 = mybir.dt.float16
F32 = mybir.dt.float32
AF = mybir.ActivationFunctionType
ALU = mybir.AluOpType

# time-tiles per sweep: (t0, N)
M_TILES = [(0, 512), (512, 512), (1024, 64)]


def build_nc(k_sweeps: int = K_SWEEPS, n_inner: int = N_INNER, unroll: int = UNROLL):
    assert k_sweeps % unroll == 0
    nc = bacc.Bacc(None, target_bir_lowering=False, debug=False)

    # ---- I/O ----
    wh_t = nc.declare_dram_parameter("wh_t", [D, 4 * D], F16, isOutput=False)
    wc_t = nc.declare_dram_parameter("wc_t", [D, D], F16, isOutput=False)
    wx_t = nc.declare_dram_parameter("wx_t", [D, 3 * D], F16, isOutput=False)
    x_t = nc.declare_dram_parameter("x_t", [D, L], F16, isOutput=False)
    bias_f = nc.declare_dram_parameter("bias_f", [128, KC], F32, isOutput=False)
    bias_i = nc.declare_dram_parameter("bias_i", [128, KC], F32, isOutput=False)
    bias_o = nc.declare_dram_parameter("bias_o", [128, KC], F32, isOutput=False)
    bias_c = nc.declare_dram_parameter("bias_c", [128, KC], F32, isOutput=False)
    hb = nc.declare_dram_parameter("hb", [128, KC, 1], F16, isOutput=False)
    cb = nc.declare_dram_parameter("cb", [128, KC, 1], F16, isOutput=False)
    ident = nc.declare_dram_parameter("ident", [128, 128], F16, isOutput=False)
    h_out = nc.declare_dram_parameter("h_out", [128, KC, L + 1], F16, isOutput=True)

    # x-contribution of f,i,o gates, [gate*8+gd, 128, L], computed in phase 1
    pre_dram = nc.dram_tensor("pre_scratch", [24, 128, L], F16)

    with tile.TileContext(nc) as tc:
        with (
            tc.tile_pool(name="const", bufs=1) as constp,
            tc.tile_pool(name="psum", bufs=8, space="PSUM") as psum,
            tc.tile_pool(name="weights", bufs=1) as wpool,
        ):
            # constants
            bf_sb = constp.tile([128, KC], F32, tag="bf")
            bi_sb = constp.tile([128, KC], F32, tag="bi")
            bo_sb = constp.tile([128, KC], F32, tag="bo")
            bc_sb = constp.tile([128, KC], F32, tag="bc")
            id_sb = constp.tile([128, 128], F16, tag="ident")
            nc.sync.dma_start(bf_sb[:, :], bias_f[:, :])
            nc.sync.dma_start(bi_sb[:, :], bias_i[:, :])
            nc.sync.dma_start(bo_sb[:, :], bias_o[:, :])
            nc.sync.dma_start(bc_sb[:, :], bias_c[:, :])
            nc.sync.dma_start(id_sb[:, :], ident[:, :])

            wh_sb = wpool.tile([128, KC, 4 * D], F16, tag="wh")
            wc_sb = wpool.tile([128, KC, D], F16, tag="wc")

            # ---------------- phase 1: pre = Wx @ x ----------------
            with tc.tile_pool(name="ph1", bufs=3) as ph1:
                xT_sb = ph1.tile([128, KC, L], F16, tag="xT")
                nc.sync.dma_start(
                    xT_sb[:, :, :], x_t[:, :].rearrange("(c p) t -> p c t", p=128)
                )
                slabs = []
                for g in range(3):
                    wx_sb = ph1.tile([128, KC, D], F16, tag="wxslab")
                    nc.sync.dma_start(
                        wx_sb[:, :, :],
                        wx_t[:, g * D:(g + 1) * D].rearrange(
                            "(c p) m -> p c m", p=128
                        ),
                    )
                    slabs.append(wx_sb)
                # recurrent weights queued after the phase-1 inputs so phase 1
                # starts immediately; the bulk load overlaps phase-1 compute
                nc.sync.dma_start(
                    wh_sb[:, :, :], wh_t[:, :].rearrange("(c p) m -> p c m", p=128)
                )
                nc.sync.dma_start(
                    wc_sb[:, :, :], wc_t[:, :].rearrange("(c p) m -> p c m", p=128)
                )
                for g in range(3):  # f, i, o
                    wx_sb = slabs[g]
                    for (t0, N) in M_TILES:
                        for gd in range(KC):
                            ps = psum.tile([128, N], F32, tag="ps")
                            for kc in range(KC):
                                nc.tensor.matmul(
                                    ps[:, :],
                                    wx_sb[:, kc, gd * 128:(gd + 1) * 128],
                                    xT_sb[:, kc, t0:t0 + N],
                                    start=(kc == 0),
                                    stop=(kc == KC - 1),
                                )
                            pre16 = ph1.tile([128, N], F16, tag="pre16")
                            nc.vector.tensor_copy(pre16[:, :], ps[:, :])
                            nc.sync.dma_start(
                                pre_dram[g * KC + gd, :, t0:t0 + N], pre16[:, :]
                            )

            # ---------------- persistent state + sweep loop ----------------
            with (
                tc.tile_pool(name="state", bufs=1) as spool,
                tc.tile_pool(name="gates", bufs=2) as gpool,
                tc.tile_pool(name="za", bufs=1) as zpool,
                tc.tile_pool(name="work", bufs=2) as wk,
                tc.tile_pool(name="prestream", bufs=4) as prepool,
            ):
                # h/c history, col j = value at local time j-1 (col 0 = boundary)
                Hf = spool.tile([128, KC, L + 1], F16, tag="H")
                Cf = spool.tile([128, KC, L + 1], F16, tag="C")
                nc.vector.memset(Hf[:, :, :], 0.0)
                nc.vector.memset(Cf[:, :, :], 0.0)
                nc.sync.dma_start(Hf[:, :, 0:1], hb[:, :, :])
                nc.sync.dma_start(Cf[:, :, 0:1], cb[:, :, :])

                def mm_group(ps, col0, t0, N, pre_tile=None):
                    """psum <- Wh[:, col0:col0+128-block] @ H (+ I @ pre)"""
                    for kc in range(KC):
                        nc.tensor.matmul(
                            ps[:, :],
                            wh_sb[:, kc, col0:col0 + 128],
                            Hf[:, kc, t0:t0 + N],
                            start=(kc == 0),
                            stop=(kc == KC - 1) and pre_tile is None,
                        )
                    if pre_tile is not None:
                        nc.tensor.matmul(
                            ps[:, :], id_sb[:, :], pre_tile[:, :],
                            start=False, stop=True,
                        )

                def sweep_body(first=False):
                    """first=True: H=C=0, so all h/c matmuls vanish — f/i/o come
                    straight from pre via ACT; skips ~70% of the sweep's matmuls
                    and hides the weight-load DMA.

                    Boundary-column deferral: each m-tile's LAST output column
                    (h/c at time t0+N-1, i.e. col t0+N) is the only value the
                    next m-tile's matmuls overlap with.  The scans/P3 write only
                    N-1 columns; the last column is emitted in the NEXT m-tile's
                    section after its matmul groups, which therefore read the
                    one-sweep-stale value (identical at the fixed point) and no
                    longer wait on this m-tile's elementwise tail.  The next
                    m-tile's scan initial and inner refinements still read the
                    fresh value (emitted before them)."""
                    deferred = None
                    for (t0, N) in M_TILES:
                        f_all = gpool.tile([128, KC, 512], F16, tag="f")
                        i_all = gpool.tile([128, KC, 512], F16, tag="i")
                        o_all = gpool.tile([128, KC, 512], F16, tag="o")
                        zA = zpool.tile([128, KC, 512], F32, tag="zA")

                        # prefetch pre tiles for this m-tile
                        pf_t, pi_t, po_t = [], [], []
                        for gd in range(KC):
                            pf = prepool.tile([128, N], F16, tag="pf")
                            nc.sync.dma_start(pf[:, :], pre_dram[gd, :, t0:t0 + N])
                            pf_t.append(pf)
                        for gd in range(KC):
                            pi = prepool.tile([128, N], F16, tag="pi")
                            nc.sync.dma_start(pi[:, :], pre_dram[KC + gd, :, t0:t0 + N])
                            pi_t.append(pi)
                        for gd in range(KC):
                            po = prepool.tile([128, N], F16, tag="po")
                            nc.sync.dma_start(po[:, :], pre_dram[2 * KC + gd, :, t0:t0 + N])
                            po_t.append(po)

                        # ---- ctilde pre-activation (h-part kept in zA) ----
                        ct_tiles = []
                        if first:
                            nc.vector.memset(zA[:, :, :], 0.0)
                        for ch in range(KC):
                            if first:
                                # H = C = 0: ctilde = tanh(b_C)
                                ct = wk.tile([128, N], F16, tag="ct")
                                nc.scalar.activation(
                                    ct[:, :], pf_t[ch][:, :], AF.Tanh,
                                    bias=bc_sb[:, ch:ch + 1], scale=0.0,
                                )
                                ct_tiles.append(ct)
                                continue
                            psA = psum.tile([128, N], F32, tag="ps")
                            mm_group(psA, 3 * D + ch * 128, t0, N)
                            nc.vector.tensor_copy(zA[:, ch, :N], psA[:, :])
                            psB = psum.tile([128, N], F32, tag="ps")
                            for kc in range(KC):
                                nc.tensor.matmul(
                                    psB[:, :],
                                    wc_sb[:, kc, ch * 128:(ch + 1) * 128],
                                    Cf[:, kc, t0:t0 + N],
                                    start=(kc == 0),
                                    stop=(kc == KC - 1),
                                )
                            tmp = wk.tile([128, N], F32, tag="tmp")
                            nc.vector.tensor_add(tmp[:, :], psB[:, :], zA[:, ch, :N])
                            ct = wk.tile([128, N], F16, tag="ct")
                            nc.scalar.activation(
                                ct[:, :], tmp[:, :], AF.Tanh, bias=bc_sb[:, ch:ch + 1]
                            )
                            ct_tiles.append(ct)

                        # ---- f and i gates ----
                        for gate_idx, (garr, pre_tiles, bias_sb) in enumerate(
                            [(f_all, pf_t, bf_sb), (i_all, pi_t, bi_sb)]
                        ):
                            for ch in range(KC):
                                if first:
                                    nc.scalar.activation(
                                        garr[:, ch, :N], pre_tiles[ch][:, :],
                                        AF.Sigmoid, bias=bias_sb[:, ch:ch + 1],
                                    )
                                    continue
                                ps = psum.tile([128, N], F32, tag="ps")
                                mm_group(ps, gate_idx * D + ch * 128, t0, N,
                                         pre_tiles[ch])
                                nc.scalar.activation(
                                    garr[:, ch, :N], ps[:, :], AF.Sigmoid,
                                    bias=bias_sb[:, ch:ch + 1],
                                )

                        # ---- flush previous m-tile's deferred boundary column
                        # (after this tile's matmul groups, before its scans) ----
                        if deferred is not None:
                            deferred()
                            deferred = None

                        # ---- scan 1: exact c given gates (N-1 cols) ----
                        for ch in range(KC):
                            b1 = wk.tile([128, N], F16, tag="b1")
                            nc.vector.tensor_mul(b1[:, :], i_all[:, ch, :N], ct_tiles[ch][:, :])
                            nc.vector.tensor_tensor_scan(
                                Cf[:, ch, t0 + 1:t0 + N],
                                f_all[:, ch, :N - 1],
                                b1[:, :N - 1],
                                Cf[:, ch, t0:t0 + 1],
                                ALU.mult,
                                ALU.add,
                            )

                        # ---- o gate (keeps PE busy while scans run) ----
                        for ch in range(KC):
                            if first:
                                nc.scalar.activation(
                                    o_all[:, ch, :N], po_t[ch][:, :], AF.Sigmoid,
                                    bias=bo_sb[:, ch:ch + 1],
                                )
                                continue
                            ps = psum.tile([128, N], F32, tag="ps")
                            mm_group(ps, 2 * D + ch * 128, t0, N, po_t[ch])
                            nc.scalar.activation(
                                o_all[:, ch, :N], ps[:, :], AF.Sigmoid,
                                bias=bo_sb[:, ch:ch + 1],
                            )

                        # ---- inner refinements of ctilde/c with updated c ----
                        blastcol = gpool.tile([128, KC], F16, tag="blc")
                        for inner_i in range(n_inner):
                            is_last = inner_i == n_inner - 1
                            ps2_tiles = []
                            for ch in range(KC):
                                ps2 = psum.tile([128, N], F32, tag="ps")
                                for kc in range(KC):
                                    nc.tensor.matmul(
                                        ps2[:, :],
                                        wc_sb[:, kc, ch * 128:(ch + 1) * 128],
                                        Cf[:, kc, t0:t0 + N],
                                        start=(kc == 0),
                                        stop=(kc == KC - 1),
                                    )
                                ps2_tiles.append(ps2)
                            for ch in range(KC):
                                tmp2 = wk.tile([128, N], F32, tag="tmp")
                                nc.vector.tensor_add(tmp2[:, :], ps2_tiles[ch][:, :], zA[:, ch, :N])
                                ct2 = wk.tile([128, N], F16, tag="ct")
                                nc.scalar.activation(
                                    ct2[:, :], tmp2[:, :], AF.Tanh, bias=bc_sb[:, ch:ch + 1]
                                )
                                b2 = wk.tile([128, N], F16, tag="b1")
                                nc.vector.tensor_mul(b2[:, :], i_all[:, ch, :N], ct2[:, :])
                                if is_last:
                                    nc.vector.tensor_copy(
                                        blastcol[:, ch:ch + 1], b2[:, N - 1:N]
                                    )
                                nc.vector.tensor_tensor_scan(
                                    Cf[:, ch, t0 + 1:t0 + N],
                                    f_all[:, ch, :N - 1],
                                    b2[:, :N - 1],
                                    Cf[:, ch, t0:t0 + 1],
                                    ALU.mult,
                                    ALU.add,
                                )

                        # ---- h = o * tanh(c) over the N-1 materialized cols ----
                        for ch in range(KC):
                            tch = wk.tile([128, N], F16, tag="tch")
                            nc.scalar.activation(
                                tch[:, :N - 1], Cf[:, ch, t0 + 1:t0 + N], AF.Tanh
                            )
                            nc.vector.tensor_mul(
                                Hf[:, ch, t0 + 1:t0 + N], o_all[:, ch, :N - 1],
                                tch[:, :N - 1],
                            )

                        # ---- deferred write of the last column (t0+N) ----
                        def make_deferred(t0=t0, N=N, f_prev=f_all, o_prev=o_all,
                                          b_prev=blastcol):
                            def flush():
                                for ch in range(KC):
                                    nc.vector.tensor_tensor_scan(
                                        Cf[:, ch, t0 + N:t0 + N + 1],
                                        f_prev[:, ch, N - 1:N],
                                        b_prev[:, ch:ch + 1],
                                        Cf[:, ch, t0 + N - 1:t0 + N],
                                        ALU.mult,
                                        ALU.add,
                                    )
                                    tc1 = wk.tile([128, 1], F16, tag="tc1")
                                    nc.scalar.activation(
                                        tc1[:, :], Cf[:, ch, t0 + N:t0 + N + 1], AF.Tanh
                                    )
                                    nc.vector.tensor_mul(
                                        Hf[:, ch, t0 + N:t0 + N + 1],
                                        o_prev[:, ch, N - 1:N], tc1[:, :],
                                    )
                            return flush

                        deferred = make_deferred()

                    # last m-tile of the sweep: flush immediately (its boundary
                    # column has no same-sweep matmul readers)
                    deferred()

                # sweep 0 (zero-state shortcut) + sweep 1 inline, rest looped
                sweep_body(first=True)
                sweep_body()
                n_loop = k_sweeps - 2
                assert n_loop % unroll == 0
                with tc.For_i(
                    0, n_loop // unroll, 1,
                    hint_engines=(
                        mybir.EngineType.PE, mybir.EngineType.DVE,
                        mybir.EngineType.Activation, mybir.EngineType.SP,
                        mybir.EngineType.Pool,
                    ),
                ):
                    for _ in range(unroll):
                        sweep_body()

                # ---------------- output ----------------
                nc.sync.dma_start(h_out[:, :, :], Hf[:, :, :])

    nc.compile()
    return nc


# ------------------------- host side -------------------------

def _prep_core_inputs(inputs):
    """Build the 8 per-core input maps from the full problem inputs."""
    x = np.asarray(inputs["target_seq"], np.float32)
    W_f = np.asarray(inputs["W_f"], np.float32)
    W_i = np.asarray(inputs["W_i"], np.float32)
    W_C = np.asarray(inputs["W_C"], np.float32)
    W_o = np.asarray(inputs["W_o"], np.float32)

    wh_t = np.concatenate(
        [W_f[:, :D].T, W_i[:, :D].T, W_o[:, :D].T, W_C[:, :D].T], axis=1
    ).astype(np.float16)                      # [D, 4D], cols = [f|i|o|C]
    wc_t = np.ascontiguousarray(W_C[:, D:].T).astype(np.float16)   # [D, D]
    wx_t = np.concatenate(
        [W_f[:, D:].T, W_i[:, D:].T, W_o[:, D:].T], axis=1
    ).astype(np.float16)                      # [D, 3D]

    def vec_pc(v):  # [D] -> [128, 8] with d = ch*128 + p
        return np.ascontiguousarray(np.asarray(v, np.float32).reshape(KC, 128).T)

    bias_f = vec_pc(inputs["b_f"])
    bias_i = vec_pc(inputs["b_i"])
    bias_o = vec_pc(inputs["b_o"])
    bias_c = vec_pc(inputs["b_C"])
    ident = np.eye(128, dtype=np.float16)

    h0 = np.asarray(inputs["encoder_h"], np.float32)
    c0 = np.asarray(inputs["encoder_c"], np.float32)

    in_maps = []
    for core in range(N_CORES):
        if core == 0:
            rows = slice(0, L)
            hbv = vec_pc(h0).astype(np.float16)[:, :, None]
            cbv = vec_pc(c0).astype(np.float16)[:, :, None]
        else:
            rows = slice(1024 * core - DELTA, 1024 * core + 1024)
            hbv = np.zeros((128, KC, 1), np.float16)
            cbv = np.zeros((128, KC, 1), np.float16)
        x_chunk_t = np.ascontiguousarray(x[rows].T).astype(np.float16)  # [D, L]
        in_maps.append({
            "wh_t": wh_t, "wc_t": wc_t, "wx_t": wx_t,
            "x_t": x_chunk_t,
            "bias_f": bias_f, "bias_i": bias_i, "bias_o": bias_o, "bias_c": bias_c,
            "hb": hbv, "cb": cbv, "ident": ident,
        })
    return in_maps


def _gather_output(results):
    """Assemble [T, D] fp32 from per-core h_out [128, 8, L+1] fp16."""
    out = np.empty((T, D), np.float32)
    for core in range(N_CORES):
        h = np.asarray(results[core]["h_out"]).reshape(128, KC, L + 1)
        # col j = h at local time j-1 ; d = ch*128 + p
        chunk = np.transpose(h, (2, 1, 0)).reshape(L + 1, D).astype(np.float32)
        if core == 0:
            out[0:1024] = chunk[1:1025]
        else:
            out[1024 * core:1024 * (core + 1)] = chunk[DELTA + 1:L + 1]
    return out


_NC_CACHE = {}


def _get_nc(k_sweeps=K_SWEEPS, n_inner=N_INNER, unroll=UNROLL):
    key = (k_sweeps, n_inner, unroll)
    if key not in _NC_CACHE:
        _NC_CACHE[key] = build_nc(k_sweeps, n_inner, unroll)
    return _NC_CACHE[key]


def kernel(**inputs) -> np.ndarray:
    nc = _get_nc()
    in_maps = _prep_core_inputs(inputs)
    res = run_bass_kernel_spmd(nc, in_maps, list(range(N_CORES)))
    return _gather_output(res.results)


if __name__ == "__main__":
    nc = build_nc()
    print("built ok")
